# revision 13
# baseline (speedup 1.0000x reference)
"""GatedGraphConvolution Trainium2 kernel (host-gather edition).

out = input + segment_sum(sigmoid(g) * e, edge_sources)
  where [g|e] = input[edge_targets] @ W.T

Sharding: edges are sharded by SOURCE node across the 8 cores (6250 nodes
each) so per-core outputs are disjoint and no collectives are needed.  The
host pre-gathers input[edge_targets] for each core's edges ("gathered rows"
per the sharding hint), sorted by 128-node source window and padded to
128-edge tiles.  The host also pre-builds the per-tile one-hot scatter
matrices (exact 0/1 values in bf16) so no engine has to materialize them.

Device per 4-tile group (512 edges):
  mm1 x4: ps[e, 0:256] = xgT_tile.T @ W.T          (PSUM group spans 2 banks)
  act:    sg = sigmoid(ps[:, :, 0:128])             (one instr / 4 tiles)
  mul:    msg = sg * ps[:, :, 128:256]              (DVE, bf16 out)
  mm2 x4: psB[srel, f] += onehot_tile.T @ msg_tile  (PSUM accum per window)
window end: y = psB + x_slice -> DRAM.

No dma_gather (SWDGE descriptor generation measured ~8ns/row on GPSIMD =
854us serial in the M-table design), no M-table round-trip, no collectives.
"""

import math
import os
import sys
from collections import deque

import numpy as np

if "/opt/trn_rl_repo" not in sys.path:
    sys.path.insert(0, "/opt/trn_rl_repo")

import ml_dtypes

P = 128  # partitions / tile edge
F = 128  # feature dim (OUT_F == IN_F == 128)
TF = 2 * F

BF16 = ml_dtypes.bfloat16

N_NODES = 50000
N_CORES = 8
NPC = N_NODES // N_CORES  # 6250
NWIN = math.ceil(NPC / P)  # 49
FP8 = ml_dtypes.float8_e4m3fn

CH = 32  # tiles per xgT/oh DMA chunk
PAIR = 4  # tiles per PSUM group (psGE [128, 1024] spans 2 banks)
MM2_LAG = 6  # tiles of lag before emitting scatter matmuls


def _plan(edge_sources: np.ndarray):
    """Per-core, per-window tile schedule. Returns (T_w, toff, TT, win_of)."""
    src = edge_sources.astype(np.int64)
    core = src // NPC
    w_all = (src % NPC) // P
    counts = np.zeros((N_CORES, NWIN), np.int64)
    np.add.at(counts, (core, w_all), 1)
    tmax = counts.max(axis=0)
    T_w = [int(math.ceil(tmax[w] / P)) for w in range(NWIN)]
    toff = np.concatenate([[0], np.cumsum(T_w)]).astype(np.int64)
    TT = int(toff[-1])
    win_of = np.zeros(TT, np.int64)
    for w in range(NWIN):
        win_of[toff[w] : toff[w + 1]] = w
    return T_w, toff, TT, win_of


def _host_core_arrays(inp16, edge_sources, edge_targets, toff, TT, c):
    """xgT [P, TT*P] bf16, oh [P, TT*P] bf16, xs [P, NWIN*F] bf16 for core c."""
    src = edge_sources.astype(np.int64)
    tgt = edge_targets.astype(np.int64)
    sel = (src // NPC) == c
    src_c = src[sel] - c * NPC
    tgt_c = tgt[sel]
    w_c = src_c // P
    srel_c = src_c % P

    order = np.argsort(w_c, kind="stable")
    w_s = w_c[order]
    srel_s = srel_c[order]
    tgt_s = tgt_c[order]

    counts = np.bincount(w_s, minlength=NWIN)
    cum = np.concatenate([[0], np.cumsum(counts)])
    within = np.arange(len(w_s)) - cum[w_s]
    pos = toff[w_s] * P + within  # slot index in [0, TT*P)

    xg = np.zeros((TT * P, F), BF16)
    xg[pos] = inp16[tgt_s]
    xgT_host = np.ascontiguousarray(xg.T)  # [P, TT*P] bf16

    ohz = np.zeros((TT * P, P), FP8)
    ohz[pos, srel_s] = 1.0
    oh_host = np.ascontiguousarray(
        ohz.reshape(TT, P, P).transpose(1, 0, 2).reshape(P, TT * P)
    )

    sl = np.zeros((NWIN * P, F), BF16)
    sl[:NPC] = inp16[c * NPC : (c + 1) * NPC]
    xs_host = np.ascontiguousarray(
        sl.reshape(NWIN, P, F).transpose(1, 0, 2).reshape(P, NWIN * F)
    )
    return xgT_host, oh_host, xs_host


def _build(T_w, toff, TT, win_of):
    import concourse.bacc as bacc
    import concourse.tile as tile
    from concourse import mybir

    nc = bacc.Bacc(
        "TRN2",
        target_bir_lowering=False,
        debug=False,
        enable_asserts=False,
        num_devices=N_CORES,
    )
    dt = mybir.dt

    xgT_d = nc.dram_tensor("xgT", [P, TT * P], dt.bfloat16, kind="ExternalInput")
    oh_d = nc.dram_tensor("oh", [P, TT * P], dt.float8e4, kind="ExternalInput")
    wT_d = nc.dram_tensor("wT", [P, TF], dt.bfloat16, kind="ExternalInput")
    xs_d = nc.dram_tensor("xs", [P, NWIN * F], dt.bfloat16, kind="ExternalInput")
    y_d = nc.dram_tensor("y", [NPC, F], dt.float32, kind="ExternalOutput")

    with tile.TileContext(nc) as tc:
        import contextlib

        with contextlib.ExitStack() as ctx:
            consts = ctx.enter_context(tc.tile_pool(name="consts", bufs=1))
            xg_pool = ctx.enter_context(tc.tile_pool(name="xg", bufs=3))
            ohc_pool = ctx.enter_context(tc.tile_pool(name="ohc", bufs=3))
            ps_pool = ctx.enter_context(tc.tile_pool(name="psGE", bufs=2, space="PSUM"))
            sg_pool = ctx.enter_context(tc.tile_pool(name="sg", bufs=3))
            msg_pool = ctx.enter_context(tc.tile_pool(name="msg", bufs=4))
            b_ps = ctx.enter_context(tc.tile_pool(name="psB", bufs=2, space="PSUM"))
            out_pool = ctx.enter_context(tc.tile_pool(name="out", bufs=2))

            wT_sb = consts.tile([P, TF], dt.bfloat16, tag="wT")
            nc.sync.dma_start(wT_sb[:], wT_d[:, :])
            xs_sb = consts.tile([P, NWIN * F], dt.bfloat16, tag="xs")
            nc.sync.dma_start(xs_sb[:], xs_d[:, :])

            state = {"chunk": None, "ohchunk": None, "c0": 0, "psB": None}
            oh_chunks = {}  # chunk idx -> (tile, start_t)
            pending = deque()  # (tile_t, msg_tile, col)

            def emit_mm2(t, msg, col):
                w = int(win_of[t])
                i = t - int(toff[w])
                if i == 0:
                    state["psB"] = b_ps.tile(
                        [P, F], dt.float32, tag="psB", name="psB"
                    )
                psB = state["psB"]
                ohc, oc0 = oh_chunks[t // CH]
                k = t - oc0
                nc.tensor.matmul(
                    psB[:],
                    lhsT=ohc[:, k * P : (k + 1) * P],
                    rhs=msg[:, col * F : (col + 1) * F],
                    start=(i == 0),
                    stop=(i == T_w[w] - 1),
                )
                if i == T_w[w] - 1:
                    rows = min(P, NPC - w * P)
                    ot = out_pool.tile([P, F], dt.float32, tag="ot")
                    nc.vector.tensor_add(
                        ot[:], psB[:], xs_sb[:, w * F : (w + 1) * F]
                    )
                    nc.gpsimd.dma_start(
                        y_d[w * P : w * P + rows, :], ot[:rows, :]
                    )

            for t0 in range(0, TT, PAIR):
                nt = min(PAIR, TT - t0)
                psGE = ps_pool.tile([P, PAIR * TF], dt.float32, tag="psGE")
                for j in range(nt):
                    t = t0 + j
                    if t % CH == 0:
                        cc = min(CH, TT - t)
                        chunk = xg_pool.tile([P, CH * P], dt.bfloat16, tag="xg")
                        nc.sync.dma_start(
                            chunk[:, : cc * P], xgT_d[:, t * P : (t + cc) * P]
                        )
                        ohchunk = ohc_pool.tile(
                            [P, CH * P], dt.float8e4, tag="ohc"
                        )
                        nc.sync.dma_start(
                            ohchunk[:, : cc * P], oh_d[:, t * P : (t + cc) * P]
                        )
                        state["chunk"] = chunk
                        state["c0"] = t
                        oh_chunks[t // CH] = (ohchunk, t)
                    k = t - state["c0"]
                    nc.tensor.matmul(
                        psGE[:, j * TF : (j + 1) * TF],
                        lhsT=state["chunk"][:, k * P : (k + 1) * P],
                        rhs=wT_sb[:],
                        start=True,
                        stop=True,
                    )
                ge = psGE[:].rearrange("p (t c) -> p t c", c=TF)
                sg = sg_pool.tile([P, PAIR * F], dt.bfloat16, tag="sg")
                nc.scalar.activation(
                    sg[:, : nt * F].rearrange("p (t c) -> p t c", c=F),
                    ge[:, :nt, 0:F],
                    mybir.ActivationFunctionType.Sigmoid,
                )
                msg = msg_pool.tile([P, PAIR * F], dt.bfloat16, tag="msg")
                nc.vector.tensor_mul(
                    msg[:, : nt * F].rearrange("p (t c) -> p t c", c=F),
                    ge[:, :nt, F:TF],
                    sg[:, : nt * F].rearrange("p (t c) -> p t c", c=F),
                )
                for j in range(nt):
                    pending.append((t0 + j, msg, j))
                while len(pending) > MM2_LAG:
                    emit_mm2(*pending.popleft())
            while pending:
                emit_mm2(*pending.popleft())

    nc.compile()
    return nc


def _in_maps(plan_arrays, W):
    wT = np.ascontiguousarray(W.T.astype(BF16))
    maps = []
    for xgT, oh, xs in plan_arrays:
        maps.append({"xgT": xgT, "oh": oh, "wT": wT, "xs": xs})
    return maps


def _install_ntff_hook():
    """Provide the antenv.axon_hooks shim trn_boot expects, so trace=True
    can capture NTFF profiles. Silently degrades if anything is missing."""
    try:
        import antenv.axon_hooks  # noqa: F401

        return
    except ImportError:
        pass
    try:
        import types

        import antenv

        mod = types.ModuleType("antenv.axon_hooks")
        _hook = [None]
        mod.set_axon_ntff_profile_hook = lambda h: _hook.__setitem__(0, h)
        mod.get_axon_ntff_profile_hook = lambda: _hook[0]
        sys.modules["antenv.axon_hooks"] = mod
        antenv.axon_hooks = mod
        from trn_agent_boot import trn_boot

        mod.set_axon_ntff_profile_hook(
            trn_boot._ntff_profile_via_ctypes("/opt/axon/libaxon_pjrt.so")
        )
    except Exception:
        pass


def kernel(**inputs) -> np.ndarray:
    inp = np.asarray(inputs["input"], np.float32)
    W = np.asarray(inputs["W"], np.float32)
    es = np.asarray(inputs["edge_sources"]).astype(np.int64)
    et = np.asarray(inputs["edge_targets"]).astype(np.int64)

    T_w, toff, TT, win_of = _plan(es)
    inp16 = inp.astype(BF16)
    plan_arrays = [
        _host_core_arrays(inp16, es, et, toff, TT, c) for c in range(N_CORES)
    ]
    nc = _build(T_w, toff, TT, win_of)

    from concourse.bass_utils import run_bass_kernel_spmd

    if bool(int(os.environ.get("GGC_TRACE", "0"))):
        _install_ntff_hook()
    res = run_bass_kernel_spmd(
        nc,
        _in_maps(plan_arrays, W),
        core_ids=list(range(N_CORES)),
        trace=bool(int(os.environ.get("GGC_TRACE", "0"))),
    )
    out = np.concatenate([res.results[c]["y"] for c in range(N_CORES)], axis=0)
    if bool(int(os.environ.get("GGC_TRACE", "0"))):
        kernel.last_results = res  # stash for test harness
    return out
